# revision 1
# baseline (speedup 1.0000x reference)
"""Deformable-conv module (offset conv -> bilinear deform conv -> sync-BN -> ReLU)
as a Trainium2 Bass kernel on 8 NeuronCores.

Sharding: core = (batch b, pixel-half ph).  Each core computes the full
256-channel output for 2048 pixels (32 image rows) of one batch image.
Full C=256 contraction is local, so no partial-sum exchange is needed;
only BN statistics cross cores (one 4KB AllReduce).

Bilinear sampling: x is host-padded into an 80x80 zero-extended grid and
packed as bf16 (value, right-neighbor) pairs in fp32 containers.  One
ap_gather index fetches both x-corners of a row; the row+1 corners come
from the same index into the grid shifted by one row.  Zero padding makes
all out-of-image corners contribute exactly 0, so no validity masks are
needed.  The 4-corner bilinear sum is folded into the deform GEMM's
contraction (each corner stream is a moving operand; PSUM accumulates).
"""
import sys, os, time

sys.path.insert(0, "/opt/trn_rl_repo")

import numpy as np
import ml_dtypes

import concourse.bacc as bacc
import concourse.tile as tile
from concourse import mybir
from concourse import library_config
from concourse.alu_op_type import AluOpType
from concourse.bass_utils import run_bass_kernel_spmd

F32 = mybir.dt.float32
BF16 = mybir.dt.bfloat16
I16 = mybir.dt.int16
AF = mybir.ActivationFunctionType
AX = mybir.AxisListType

B, C, H, W, O = 4, 256, 64, 64, 256
K = 3
K2 = 9
EPS = 1e-5
PADE = 8          # extension pad on each side of the image
EG = H + 2 * PADE  # 80: extended grid edge
EGF = EG * EG      # 6400 ext pixels
PREB = 648         # leading zero rows so biased indices need no -648
XTROWS = PREB + EGF  # 7048 rows in the HBM gather table
NELEM = 6967       # gather source rows (covers max biased index 6966)
MAGIC = 12582912.0  # 1.5 * 2**23: fp32 round-to-int trick
HALF = 2048        # pixels per core
HROWS = HALF // W  # 32 image rows per core
NCORES = 8
NTOT = B * H * W   # BN normalization count


USE_DMA_GATHER = os.environ.get("DC_DMA_GATHER", "0") == "1"


def build_program(num_devices=NCORES):
    nc = bacc.Bacc("TRN2", target_bir_lowering=False, debug=False,
                   num_devices=num_devices, num_swdge_queues=4)

    if USE_DMA_GATHER:
        xb_d = nc.dram_tensor("xb", [2, 128, EGF], BF16,
                              kind="ExternalInput").ap()
        xt_d = nc.dram_tensor("xt", [XTROWS, 512], BF16,
                              kind="ExternalInput").ap()
    else:
        xp_d = nc.dram_tensor("xp", [2, 128, XTROWS + EG], F32,
                              kind="ExternalInput").ap()
    woff_d = nc.dram_tensor("woff", [2, 128, K2, 18], BF16, kind="ExternalInput").ap()
    wdef_d = nc.dram_tensor("wdef", [2, 128, K2, O], BF16, kind="ExternalInput").ap()
    kb_d = nc.dram_tensor("kb", [K2, 2, HALF], F32, kind="ExternalInput").ap()
    gb_d = nc.dram_tensor("gb", [2, 128, 2], F32, kind="ExternalInput").ap()
    out_d = nc.dram_tensor("out", [O, HALF], F32, kind="ExternalOutput").ap()
    # internal DRAM scratch for layout bounces
    widx_d = nc.dram_tensor("widx_s", [K2, HALF], I16).ap()
    wmap_d = nc.dram_tensor("wmap_s", [2 * K2, 2, HALF], BF16).ap()
    stats_in_d = nc.dram_tensor("stats_in", [128, 4], F32).ap()
    stats_out_d = nc.dram_tensor("stats_out", [128, 4], F32, addr_space="Shared").ap()

    with tile.TileContext(nc) as tc:
        with tc.tile_pool(name="per", bufs=1) as per, \
             tc.tile_pool(name="wb", bufs=2) as wbp, \
             tc.tile_pool(name="gt", bufs=2) as gtp, \
             tc.tile_pool(name="tt", bufs=2) as ttp, \
             tc.tile_pool(name="ps", bufs=2, space="PSUM") as psp:

            # ---- inputs to SBUF ----
            if USE_DMA_GATHER:
                XB0 = per.tile([128, EGF], BF16, tag="xb0")
                XB1 = per.tile([128, EGF], BF16, tag="xb1")
                nc.sync.dma_start(XB0[:], xb_d[0])
                nc.sync.dma_start(XB1[:], xb_d[1])
                XB = [XB0, XB1]
            else:
                nc.gpsimd.load_library(library_config.ap_gather)
                XP0 = per.tile([128, XTROWS + EG], F32, tag="xp0")
                XP1 = per.tile([128, XTROWS + EG], F32, tag="xp1")
                nc.sync.dma_start(XP0[:], xp_d[0])
                nc.sync.dma_start(XP1[:], xp_d[1])
                XP = [XP0, XP1]
            WOFF = per.tile([128, 2, K2, 18], BF16, tag="woff")
            nc.sync.dma_start(WOFF[:, 0], woff_d[0])
            nc.sync.dma_start(WOFF[:, 1], woff_d[1])
            WDEF = per.tile([128, 2, K2, O], BF16, tag="wdef")
            nc.sync.dma_start(WDEF[:, 0], wdef_d[0])
            nc.sync.dma_start(WDEF[:, 1], wdef_d[1])
            GBt = per.tile([128, 2, 2], F32, tag="gb")
            nc.sync.dma_start(GBt[:, 0], gb_d[0])
            nc.sync.dma_start(GBt[:, 1], gb_d[1])

            # [part, 80 ext rows, 80 cols] view for conv windows
            if USE_DMA_GATHER:
                xv = [XB[cg].rearrange("p (r c) -> p r c", c=EG)
                      for cg in range(2)]
                xstep = 1
            else:
                # stride-2 bf16 view into the container tiles (element 2i = xe[i])
                xv = [XP[cg][:, PREB:PREB + EGF].bitcast(BF16)
                      .rearrange("p (r c) -> p r c", c=2 * EG) for cg in range(2)]
                xstep = 2

            # ---- offset conv: two GEMMs (y comps, x comps) ----
            # psum partitions = 9 taps; moving = 32x64 image window, 512-col chunks
            ps_y = psp.tile([K2, HALF], F32, tag="ps")
            ps_x = psp.tile([K2, HALF], F32, tag="ps")
            n_mm = 0
            for comp, pst in ((0, ps_y), (1, ps_x)):
                for cg in range(2):
                    for kk in range(K2):
                        dy, dx = kk // K - 1, kk % K - 1
                        r0 = PADE + dy   # ext row of first window row (ph folded in kb? no: rows differ per core!)
                        # NOTE: per-core row offset handled via kb; but the
                        # window itself must read this core's rows.  The row
                        # base depends on ph which is NOT known at build time,
                        # so we bake it via a per-core DRAM input instead?  No:
                        # all cores run the same program; we make the window
                        # row base a *program constant* = PADE+dy and add the
                        # per-core 32-row offset by shifting the data on host?
                        # Host shifts: xp is the full 80x80 grid; instead the
                        # moving AP below uses rows [PADE+dy+ROWOFF ...] with
                        # ROWOFF supplied via host-rolled kb... -> resolved by
                        # building xp PER CORE with the core's 32-row window
                        # centered: see host prep (xe rolled so that rows
                        # [PADE..PADE+32) are this core's rows).
                        stat = WOFF[:, cg, kk, 9 * comp:9 * comp + 9]
                        for q in range(4):
                            rq = r0 + 8 * q
                            mov = xv[cg][:, rq:rq + 8,
                                         xstep * (PADE + dx):
                                         xstep * (PADE + dx) + xstep * W:xstep]
                            nc.tensor.matmul(
                                pst[:, 512 * q:512 * (q + 1)], stat, mov,
                                start=(cg == 0 and kk == 0),
                                stop=(cg == 1 and kk == K2 - 1))
                            n_mm += 1

            # ---- coordinate math ----
            # A = py + 15.5 (kb holds tap offset + base + 15.5); add conv psums
            AYX = per.tile([K2, 2, HALF], F32, tag="ayx")
            nc.sync.dma_start(AYX[:], kb_d[:])
            nc.vector.tensor_tensor(AYX[:, 0], AYX[:, 0], ps_y[:], AluOpType.add)
            nc.vector.tensor_tensor(AYX[:, 1], AYX[:, 1], ps_x[:], AluOpType.add)
            # FI = round(A) = floor(py) + 16 (fp32 magic-number round)
            FI = per.tile([K2, 2, HALF], F32, tag="fi")
            nc.vector.tensor_scalar(FI[:], AYX[:], MAGIC, -MAGIC,
                                    AluOpType.add, AluOpType.add)
            # D = A - FI in (-0.5, 0.5]; frac = D + 0.5, 1-frac = 0.5 - D
            nc.vector.tensor_tensor(AYX[:], AYX[:], FI[:], AluOpType.subtract)
            # clip FI to [8, 86] (= coord in [-8, 70]), in place
            nc.vector.tensor_scalar(FI[:], FI[:], 8.0, 86.0,
                                    AluOpType.max, AluOpType.min)
            # gather index = 80*ycl + xcl (maps into the PREB-padded grid)
            nc.vector.scalar_tensor_tensor(FI[:, 1], FI[:, 0], 80.0, FI[:, 1],
                                           AluOpType.mult, AluOpType.add)
            WIDX16 = per.tile([K2, HALF], I16, tag="widx16")
            nc.vector.tensor_copy(WIDX16[:], FI[:, 1])
            # OM = 0.5 - D = 1 - frac
            OM = per.tile([K2, 2, HALF], F32, tag="om")
            nc.vector.tensor_scalar(OM[:], AYX[:], -1.0, 0.5,
                                    AluOpType.mult, AluOpType.add)
            # FI[:,1] now free: reuse for frac_x = D_x + 0.5
            nc.vector.tensor_scalar(FI[:, 1], AYX[:, 1], 0.5, None, AluOpType.add)
            # corner-major weight maps [k, x-corner, pixel], both tiles on
            # partitions 0-8 (DVE lanes are partition-locked): WMA=row0, WMB=row1
            WMA = per.tile([K2, 2, HALF], BF16, tag="wma")
            WMB = per.tile([K2, 2, HALF], BF16, tag="wmb")
            nc.vector.tensor_tensor(WMA[:, 0], OM[:, 0], OM[:, 1], AluOpType.mult)
            nc.vector.tensor_tensor(WMA[:, 1], OM[:, 0], FI[:, 1], AluOpType.mult)
            nc.vector.scalar_tensor_tensor(WMB[:, 0], AYX[:, 0], 0.5, OM[:, 1],
                                           AluOpType.add, AluOpType.mult)
            nc.vector.scalar_tensor_tensor(WMB[:, 1], AYX[:, 0], 0.5, FI[:, 1],
                                           AluOpType.add, AluOpType.mult)

            # ---- DRAM bounce: wrap indices for ap_gather, broadcast weights ----
            nc.sync.dma_start(widx_d[:], WIDX16[:])
            nc.sync.dma_start(wmap_d[0:K2], WMA[:])
            nc.sync.dma_start(wmap_d[K2:], WMB[:])
            WIDXW = per.tile([128, K2, HALF // 16], I16, tag="widxw")
            widx_r = widx_d.rearrange("k (c s) -> s k c", s=16)
            for g in range(8):
                nc.sync.dma_start(WIDXW[16 * g:16 * (g + 1)], widx_r)

            # ---- main loop: gather, weight, GEMM-accumulate ----
            PSD0 = psp.tile([128, HALF], F32, tag="ps")
            PSD1 = psp.tile([128, HALF], F32, tag="ps")
            PSD = [PSD0, PSD1]
            for kk in range(K2):
                for row in range(2):
                    if USE_DMA_GATHER:
                        # weights: e-slots 0,1 = x-corner0 (cg0, cg1); 2,3 = x-corner1
                        WB = wbp.tile([128, 4, HALF], BF16, tag="wb")
                        for xc in range(2):
                            wsrc = wmap_d[K2 * row + kk, xc]\
                                .unsqueeze(0).unsqueeze(0)
                            nc.sync.dma_start(WB[:, 2 * xc:2 * xc + 2],
                                              wsrc.broadcast_to((128, 2, HALF)))
                        # one gather fetches all 4 (c-group, x-corner) streams
                        G = gtp.tile([128, 4, HALF], BF16, tag="g")
                        nc.gpsimd.dma_gather(
                            G[:], xt_d[EG * row:EG * row + NELEM],
                            WIDXW[:, kk], num_idxs=HALF, num_idxs_reg=HALF,
                            elem_size=512, transpose=True,
                            queue_num=(2 * kk + row) % 4)
                        T = ttp.tile([128, 4, HALF], BF16, tag="t")
                        nc.vector.tensor_tensor(T[:], G[:], WB[:], AluOpType.mult)
                        first = (kk == 0 and row == 0)
                        last = (kk == K2 - 1 and row == 1)
                        for cg in range(2):
                            for oh in range(2):
                                stat = WDEF[:, cg, kk, 128 * oh:128 * (oh + 1)]
                                for par in range(2):
                                    for q in range(4):
                                        mov = T[:, 2 * par + cg,
                                                512 * q:512 * (q + 1)]
                                        nc.tensor.matmul(
                                            PSD[oh][:, 512 * q:512 * (q + 1)],
                                            stat, mov,
                                            start=(first and cg == 0 and par == 0),
                                            stop=(last and cg == 1 and par == 1))
                        continue
                    # ---- ap_gather path (x-corner pairs packed in containers) ----
                    WB = wbp.tile([128, 2 * HALF], BF16, tag="wb")
                    wsrc = wmap_d[K2 * row + kk].unsqueeze(0)
                    # interleave (x0,x1) weights per pixel via strided dst
                    nc.sync.dma_start(WB[:, 0::2],
                                      wmap_d[K2 * row + kk, 0].unsqueeze(0)
                                      .broadcast_to((128, HALF)))
                    nc.sync.dma_start(WB[:, 1::2],
                                      wmap_d[K2 * row + kk, 1].unsqueeze(0)
                                      .broadcast_to((128, HALF)))
                    for cg in range(2):
                        G = gtp.tile([128, HALF], F32, tag="g")
                        nc.gpsimd.ap_gather(
                            G[:], XP[cg][:, EG * row:EG * row + NELEM + 33],
                            WIDXW[:, kk], channels=128, num_elems=NELEM + 33,
                            d=1, num_idxs=HALF)
                        T = ttp.tile([128, 2 * HALF], BF16, tag="t")
                        nc.vector.tensor_tensor(T[:], G.bitcast(BF16)[:], WB[:],
                                                AluOpType.mult)
                        first = (kk == 0 and row == 0 and cg == 0)
                        last = (kk == K2 - 1 and row == 1 and cg == 1)
                        for oh in range(2):
                            stat = WDEF[:, cg, kk, 128 * oh:128 * (oh + 1)]
                            for par in range(2):
                                Tp = T[:, par::2]
                                for q in range(4):
                                    mov = Tp[:, 512 * q:512 * (q + 1)]
                                    nc.tensor.matmul(
                                        PSD[oh][:, 512 * q:512 * (q + 1)],
                                        stat, mov,
                                        start=(first and par == 0),
                                        stop=(last and par == 1))

            # ---- BN stats + AllReduce ----
            SM = per.tile([128, 48], F32, tag="sm")
            TRASH = ttp.tile([128, 2 * HALF], BF16, tag="t")
            ZERO = SM[:, 40:41]
            EPSAP = SM[:, 41:42]
            nc.vector.memset(ZERO, 0.0)
            nc.vector.memset(EPSAP, float(EPS))
            for oh in range(2):
                nc.vector.tensor_reduce(SM[:, oh:oh + 1], PSD[oh][:],
                                        AX.X, AluOpType.add)
                nc.scalar.activation(TRASH[:, 0:HALF], PSD[oh][:], AF.Square,
                                     bias=ZERO, accum_out=SM[:, 2 + oh:3 + oh])
            nc.sync.dma_start(stats_in_d[:], SM[:, 0:4])
            nc.gpsimd.collective_compute(
                "AllReduce", AluOpType.add,
                replica_groups=[list(range(num_devices))],
                ins=[stats_in_d[:]], outs=[stats_out_d[:]])
            nc.sync.dma_start(SM[:, 8:12], stats_out_d[:])

            for oh in range(2):
                mean = SM[:, 16 + oh:17 + oh]
                ex2 = SM[:, 18 + oh:19 + oh]
                var = SM[:, 20 + oh:21 + oh]
                sd = SM[:, 22 + oh:23 + oh]
                rstd = SM[:, 24 + oh:25 + oh]
                s1 = SM[:, 26 + oh:27 + oh]
                ms = SM[:, 28 + oh:29 + oh]
                s2 = SM[:, 30 + oh:31 + oh]
                nc.vector.tensor_scalar(mean, SM[:, 8 + oh:9 + oh],
                                        1.0 / NTOT, None, AluOpType.mult)
                nc.vector.tensor_scalar(ex2, SM[:, 10 + oh:11 + oh],
                                        1.0 / NTOT, None, AluOpType.mult)
                nc.vector.tensor_tensor(var, mean, mean, AluOpType.mult)
                nc.vector.tensor_tensor(var, ex2, var, AluOpType.subtract)
                nc.scalar.activation(sd, var, AF.Sqrt, bias=EPSAP)
                nc.vector.reciprocal(rstd, sd)
                nc.vector.tensor_tensor(s1, GBt[:, oh, 0:1], rstd, AluOpType.mult)
                nc.vector.tensor_tensor(ms, mean, s1, AluOpType.mult)
                nc.vector.tensor_tensor(s2, GBt[:, oh, 1:2], ms, AluOpType.subtract)
                OUTS = gtp.tile([128, HALF], F32, tag="g")
                nc.scalar.activation(OUTS[:], PSD[oh][:], AF.Relu,
                                     bias=s2, scale=s1)
                nc.sync.dma_start(out_d[128 * oh:128 * (oh + 1), :], OUTS[:])

    nc.compile()
    return nc


def host_inputs(x, w_off, b_off, w_def, gamma, beta):
    """Build the 8 per-core input dicts."""
    x = np.asarray(x, np.float32)
    w_off = np.asarray(w_off, np.float32)
    b_off = np.asarray(b_off, np.float32)
    w_def = np.asarray(w_def, np.float32)
    gamma = np.asarray(gamma, np.float32)
    beta = np.asarray(beta, np.float32)

    # weight stationaries, shared by all cores.
    # woff[cg, c, kk, j]: off-conv stationary for kernel position kk; output
    # column j<9 = tap j's dy channel (2j), j>=9 = tap (j-9)'s dx channel.
    woff = np.zeros((2, 128, K2, 18), np.float32)
    wdef = np.zeros((2, 128, K2, O), np.float32)
    for cg in range(2):
        cs = slice(128 * cg, 128 * (cg + 1))
        for kk in range(K2):
            ky, kx = kk // K, kk % K
            for j in range(K2):
                woff[cg, :, kk, j] = w_off[2 * j, cs, ky, kx]
                woff[cg, :, kk, 9 + j] = w_off[2 * j + 1, cs, ky, kx]
            wdef[cg, :, kk, :] = w_def[:, cs, ky, kx].T
    woff = woff.astype(ml_dtypes.bfloat16)
    wdef = wdef.astype(ml_dtypes.bfloat16)

    gb = np.zeros((2, 128, 2), np.float32)
    gb[0, :, 0], gb[1, :, 0] = gamma[:128], gamma[128:]
    gb[0, :, 1], gb[1, :, 1] = beta[:128], beta[128:]

    in_maps = []
    for core in range(NCORES):
        b, ph = core // 2, core % 2
        # extended zero-padded grid, rolled so ext rows [0..80) cover this
        # core's rows: ext row r corresponds to image row r - 8 + 32*ph
        xe = np.zeros((C, EG, EG), np.float32)
        r_lo, r_hi = 32 * ph - PADE, 32 * ph - PADE + EG
        s_lo, s_hi = max(0, r_lo), min(H, r_hi)
        xe[:, s_lo - r_lo:s_hi - r_lo, PADE:PADE + W] = x[b, :, s_lo:s_hi, :]
        xb = xe.reshape(2, 128, EGF).astype(ml_dtypes.bfloat16)
        # container pack for the ap_gather path: element i = (xe[i], xe[i+1])
        xcols = XTROWS + EG
        flatc = np.zeros((C, xcols + 1), ml_dtypes.bfloat16)
        flatc[:, PREB:PREB + EGF] = xe.reshape(C, EGF)
        lo = flatc[:, :xcols].view(np.uint16).astype(np.uint32)
        hi = flatc[:, 1:xcols + 1].view(np.uint16).astype(np.uint32)
        xp = (lo | (hi << np.uint32(16))).view(np.float32).reshape(2, 128, xcols)
        # HBM gather row table: row j = [cg0 pix j, cg1 pix j, cg0 j+1, cg1 j+1]
        flat = np.zeros((C, XTROWS + 1), ml_dtypes.bfloat16)
        flat[:, PREB:PREB + EGF] = xe.reshape(C, EGF)
        xt = np.empty((XTROWS, 512), ml_dtypes.bfloat16)
        xt[:, 0:128] = flat[0:128, :XTROWS].T
        xt[:, 128:256] = flat[128:256, :XTROWS].T
        xt[:, 256:384] = flat[0:128, 1:XTROWS + 1].T
        xt[:, 384:512] = flat[128:256, 1:XTROWS + 1].T

        # kb[k, 0, p] = 16 + (ky-1) + h_local(p) + b_off_y ; h_local = p//64 + ...
        # NOTE: the gather/window row coords are *local* to the rolled grid:
        # local row of pixel p is p//64 (0..31), plus the conv sampling is
        # relative; py_local = off_y + (ky-1) + p//64.  The +16 mod-floor bias.
        kb = np.zeros((K2, 2, HALF), np.float32)
        pl = np.arange(HALF, dtype=np.float32)
        hloc = np.floor(pl / W)
        wloc = pl % W
        for kk in range(K2):
            ky, kx = kk // K, kk % K
            kb[kk, 0, :] = 15.5 + (ky - 1) + hloc + b_off[2 * kk]
            kb[kk, 1, :] = 15.5 + (kx - 1) + wloc + b_off[2 * kk + 1]
        m = {"woff": np.asarray(woff), "wdef": np.asarray(wdef),
             "kb": kb, "gb": gb}
        if USE_DMA_GATHER:
            m["xb"] = xb
            m["xt"] = xt
        else:
            m["xp"] = xp
        in_maps.append(m)
    return in_maps


_prog_cache = {}


def _get_prog():
    if "nc" not in _prog_cache:
        _prog_cache["nc"] = build_program(NCORES)
    return _prog_cache["nc"]


def kernel(x, w_off, b_off, w_def, gamma, beta):
    nc = _get_prog()
    in_maps = host_inputs(x, w_off, b_off, w_def, gamma, beta)
    res = run_bass_kernel_spmd(nc, in_maps, core_ids=list(range(NCORES)))
    out = np.zeros((B, O, H, W), np.float32)
    for core in range(NCORES):
        b, ph = core // 2, core % 2
        out[b, :, 32 * ph:32 * (ph + 1), :] = \
            res.results[core]["out"].reshape(O, HROWS, W)
    return out



# revision 10
# speedup vs baseline: 5.2712x; 5.2712x over previous
"""Deformable-conv module (offset conv -> bilinear deform conv -> sync-BN -> ReLU)
as a Trainium2 Bass kernel on 8 NeuronCores.

Sharding: core = (batch b, pixel-half ph).  Each core computes the full
256-channel output for 2048 pixels (32 image rows) of one batch image.
Full C=256 contraction is local, so no partial-sum exchange is needed;
only BN statistics cross cores (one 4KB AllReduce).

Bilinear sampling: x is host-padded into an 80x80 zero-extended grid and
packed as bf16 (value, right-neighbor) pairs in fp32 containers.  One
ap_gather index fetches both x-corners of a row; the row+1 corners come
from the same index into the grid shifted by one row.  Zero padding makes
all out-of-image corners contribute exactly 0, so no validity masks are
needed.  The 4-corner bilinear sum is folded into the deform GEMM's
contraction (each corner stream is a moving operand; PSUM accumulates).
"""
import sys, os, time

sys.path.insert(0, "/opt/trn_rl_repo")

import numpy as np
import ml_dtypes

import concourse.bacc as bacc
import concourse.tile as tile
from concourse import mybir
from concourse import library_config
from concourse.alu_op_type import AluOpType
from concourse.bass_utils import run_bass_kernel_spmd

F32 = mybir.dt.float32
BF16 = mybir.dt.bfloat16
I16 = mybir.dt.int16
AF = mybir.ActivationFunctionType
AX = mybir.AxisListType
MM = mybir.MatmulPerfMode

B, C, H, W, O = 4, 256, 64, 64, 256
K = 3
K2 = 9
EPS = 1e-5
PADE = 8          # extension pad on each side of the image
EG = H + 2 * PADE  # 80: extended grid edge
EGF = EG * EG      # 6400 ext pixels
PREB = 648         # leading zero rows so biased indices need no -648
XTROWS = PREB + EGF  # 7048 rows in the HBM gather table
NELEM = 6967       # gather source rows (covers max biased index 6966)
MAGIC = 12582912.0  # 1.5 * 2**23: fp32 round-to-int trick
HALF = 2048        # pixels per core
HROWS = HALF // W  # 32 image rows per core
NCORES = 8
NTOT = B * H * W   # BN normalization count


def build_program(num_devices=NCORES):
    nc = bacc.Bacc("TRN2", target_bir_lowering=False, debug=False,
                   num_devices=num_devices, num_swdge_queues=4)

    xp_d = nc.dram_tensor("xp", [2, 128, XTROWS + EG], F32,
                          kind="ExternalInput").ap()
    woff_d = nc.dram_tensor("woff", [2, 128, K2, 18], BF16, kind="ExternalInput").ap()
    wdef_d = nc.dram_tensor("wdef", [2, 128, K2, O], BF16, kind="ExternalInput").ap()
    kb_d = nc.dram_tensor("kb", [K2, 2, HALF], F32, kind="ExternalInput").ap()
    gb_d = nc.dram_tensor("gb", [2, 128, 2], F32, kind="ExternalInput").ap()
    out_d = nc.dram_tensor("out", [O, HALF], F32, kind="ExternalOutput").ap()
    # internal DRAM scratch for layout bounces
    widx_d = nc.dram_tensor("widx_s", [K2, HALF], I16).ap()
    wmap_d = nc.dram_tensor("wmap_s", [K2, 2, HALF], F32).ap()
    stats_in_d = nc.dram_tensor("stats_in", [128, 4], F32).ap()
    stats_out_d = nc.dram_tensor("stats_out", [128, 4], F32, addr_space="Shared").ap()

    with tile.TileContext(nc) as tc:
        with tc.tile_pool(name="per", bufs=1) as per, \
             tc.tile_pool(name="wb", bufs=2) as wbp, \
             tc.tile_pool(name="gt", bufs=2) as gtp, \
             tc.tile_pool(name="tt", bufs=2) as ttp, \
             tc.tile_pool(name="ps", bufs=2, space="PSUM") as psp:

            # ---- inputs to SBUF ----
            nc.gpsimd.load_library(library_config.ap_gather)
            XP0 = per.tile([128, XTROWS + EG], F32, tag="xp0")
            XP1 = per.tile([128, XTROWS + EG], F32, tag="xp1")
            nc.sync.dma_start(XP0[:], xp_d[0])
            nc.sync.dma_start(XP1[:], xp_d[1])
            XP = [XP0, XP1]
            WOFF = per.tile([128, 2, K2, 18], BF16, tag="woff")
            nc.sync.dma_start(WOFF[:, 0], woff_d[0])
            nc.sync.dma_start(WOFF[:, 1], woff_d[1])
            WDEF = per.tile([128, 2, K2, O], BF16, tag="wdef")
            nc.sync.dma_start(WDEF[:, 0], wdef_d[0])
            nc.sync.dma_start(WDEF[:, 1], wdef_d[1])
            GBt = per.tile([128, 2, 2], F32, tag="gb")
            nc.sync.dma_start(GBt[:, 0], gb_d[0])
            nc.sync.dma_start(GBt[:, 1], gb_d[1])

            # [part, 80 ext rows, 80 cols] view for conv windows
            # stride-2 bf16 view into the container tiles (element 2i = xe[i])
            xv = [XP[cg][:, PREB:PREB + EGF].bitcast(BF16)
                  .rearrange("p (r c) -> p r c", c=2 * EG) for cg in range(2)]
            xstep = 2

            # ---- offset conv: two GEMMs (y comps, x comps) ----
            # psum partitions = 9 taps; moving = 32x64 image window, 512-col chunks
            ps_y = psp.tile([K2, HALF], F32, tag="ps")
            ps_x = psp.tile([K2, HALF], F32, tag="ps")
            n_mm = 0
            for comp, pst in ((0, ps_y), (1, ps_x)):
                for cg in range(2):
                    for kk in range(K2):
                        dy, dx = kk // K - 1, kk % K - 1
                        r0 = PADE + dy   # ext row of first window row (ph folded in kb? no: rows differ per core!)
                        # NOTE: per-core row offset handled via kb; but the
                        # window itself must read this core's rows.  The row
                        # base depends on ph which is NOT known at build time,
                        # so we bake it via a per-core DRAM input instead?  No:
                        # all cores run the same program; we make the window
                        # row base a *program constant* = PADE+dy and add the
                        # per-core 32-row offset by shifting the data on host?
                        # Host shifts: xp is the full 80x80 grid; instead the
                        # moving AP below uses rows [PADE+dy+ROWOFF ...] with
                        # ROWOFF supplied via host-rolled kb... -> resolved by
                        # building xp PER CORE with the core's 32-row window
                        # centered: see host prep (xe rolled so that rows
                        # [PADE..PADE+32) are this core's rows).
                        stat = WOFF[:, cg, kk, 9 * comp:9 * comp + 9]
                        for q in range(4):
                            rq = r0 + 8 * q
                            mov = xv[cg][:, rq:rq + 8,
                                         xstep * (PADE + dx):
                                         xstep * (PADE + dx) + xstep * W:xstep]
                            nc.tensor.matmul(
                                pst[:, 512 * q:512 * (q + 1)], stat, mov,
                                start=(cg == 0 and kk == 0),
                                stop=(cg == 1 and kk == K2 - 1))
                            n_mm += 1

            # ---- coordinate math ----
            # A = py + 15.5 (kb holds tap offset + base + 15.5); add conv psums
            AYX = per.tile([K2, 2, HALF], F32, tag="ayx")
            nc.sync.dma_start(AYX[:], kb_d[:])
            nc.vector.tensor_tensor(AYX[:, 0], AYX[:, 0], ps_y[:], AluOpType.add)
            nc.vector.tensor_tensor(AYX[:, 1], AYX[:, 1], ps_x[:], AluOpType.add)
            # FI = round(A) = floor(py) + 16 (fp32 magic-number round)
            FI = per.tile([K2, 2, HALF], F32, tag="fi")
            nc.vector.tensor_scalar(FI[:], AYX[:], MAGIC, -MAGIC,
                                    AluOpType.add, AluOpType.add)
            # D = A - FI in (-0.5, 0.5]; frac = D + 0.5, 1-frac = 0.5 - D
            nc.vector.tensor_tensor(AYX[:], AYX[:], FI[:], AluOpType.subtract)
            # clip FI to [8, 86] (= coord in [-8, 70]), in place
            nc.vector.tensor_scalar(FI[:], FI[:], 8.0, 86.0,
                                    AluOpType.max, AluOpType.min)
            # gather index = 80*ycl + xcl (maps into the PREB-padded grid)
            nc.vector.scalar_tensor_tensor(FI[:, 1], FI[:, 0], 80.0, FI[:, 1],
                                           AluOpType.mult, AluOpType.add)
            WIDX16 = per.tile([K2, HALF], I16, tag="widx16")
            nc.vector.tensor_copy(WIDX16[:], FI[:, 1])
            # OM = 0.5 - D = 1 - frac
            OM = per.tile([K2, 2, HALF], F32, tag="om")
            nc.vector.tensor_scalar(OM[:], AYX[:], -1.0, 0.5,
                                    AluOpType.mult, AluOpType.add)
            # FI[:,1] now free: reuse for frac_x = D_x + 0.5
            nc.vector.tensor_scalar(FI[:, 1], AYX[:, 1], 0.5, None, AluOpType.add)
            # pixel-interleaved weight maps [k, row, (pixel, x-corner)] so the
            # per-iteration broadcast DMA is contiguous; the bf16 (w_x0, w_x1)
            # pair of a pixel shares one fp32 container word
            WMI = per.tile([K2, 2, 2 * HALF], BF16, tag="wmi")
            nc.vector.tensor_tensor(WMI[:, 0, 0::2], OM[:, 0], OM[:, 1],
                                    AluOpType.mult)
            nc.vector.tensor_tensor(WMI[:, 0, 1::2], OM[:, 0], FI[:, 1],
                                    AluOpType.mult)
            nc.vector.scalar_tensor_tensor(WMI[:, 1, 0::2], AYX[:, 0], 0.5,
                                           OM[:, 1], AluOpType.add,
                                           AluOpType.mult)
            nc.vector.scalar_tensor_tensor(WMI[:, 1, 1::2], AYX[:, 0], 0.5,
                                           FI[:, 1], AluOpType.add,
                                           AluOpType.mult)

            # ---- DRAM bounce: wrap indices for ap_gather, broadcast weights ----
            nc.sync.dma_start(widx_d[:], WIDX16[:])
            nc.sync.dma_start(wmap_d[:], WMI.bitcast(F32)[:])
            WIDXW = per.tile([128, K2, HALF // 16], I16, tag="widxw")
            widx_r = widx_d.rearrange("k (c s) -> s k c", s=16)
            for g in range(8):
                nc.sync.dma_start(WIDXW[16 * g:16 * (g + 1)], widx_r)

            # ---- main loop: gather, weight, GEMM-accumulate ----
            PSD0 = psp.tile([128, HALF], F32, tag="ps")
            PSD1 = psp.tile([128, HALF], F32, tag="ps")
            PSD = [PSD0, PSD1]
            for kk in range(K2):
                for row in range(2):
                    # contiguous broadcast of the pixel-interleaved pair weights
                    WB = wbp.tile([128, HALF], F32, tag="wb")
                    nc.sync.dma_start(WB[:], wmap_d[kk, row].unsqueeze(0)
                                      .broadcast_to((128, HALF)))
                    for cg in range(2):
                        G = gtp.tile([128, HALF], F32, tag="g")
                        nc.gpsimd.ap_gather(
                            G[:], XP[cg][:, EG * row:EG * row + NELEM + 33],
                            WIDXW[:, kk], channels=128, num_elems=NELEM + 33,
                            d=1, num_idxs=HALF)
                        T = ttp.tile([128, 2 * HALF], BF16, tag="t")
                        nc.vector.tensor_tensor(T[:], G.bitcast(BF16)[:],
                                                WB.bitcast(BF16)[:],
                                                AluOpType.mult)
                        first = (kk == 0 and row == 0 and cg == 0)
                        last = (kk == K2 - 1 and row == 1 and cg == 1)
                        for oh in range(2):
                            stat = WDEF[:, cg, kk, 128 * oh:128 * (oh + 1)]
                            for par in range(2):
                                Tp = T[:, par::2]
                                for q in range(4):
                                    mov = Tp[:, 512 * q:512 * (q + 1)]
                                    nc.tensor.matmul(
                                        PSD[oh][:, 512 * q:512 * (q + 1)],
                                        stat, mov,
                                        start=(first and par == 0),
                                        stop=(last and par == 1))

            # ---- BN stats + AllReduce ----
            SM = per.tile([128, 48], F32, tag="sm")
            TRASH = ttp.tile([128, 2 * HALF], BF16, tag="t")
            ZERO = SM[:, 40:41]
            EPSAP = SM[:, 41:42]
            nc.vector.memset(ZERO, 0.0)
            nc.vector.memset(EPSAP, float(EPS))
            for oh in range(2):
                nc.vector.tensor_reduce(SM[:, oh:oh + 1], PSD[oh][:],
                                        AX.X, AluOpType.add)
                nc.scalar.activation(TRASH[:, 0:HALF], PSD[oh][:], AF.Square,
                                     bias=ZERO, accum_out=SM[:, 2 + oh:3 + oh])
            nc.sync.dma_start(stats_in_d[:], SM[:, 0:4])
            nc.gpsimd.collective_compute(
                "AllReduce", AluOpType.add,
                replica_groups=[list(range(num_devices))],
                ins=[stats_in_d[:]], outs=[stats_out_d[:]])
            nc.sync.dma_start(SM[:, 8:12], stats_out_d[:])

            for oh in range(2):
                mean = SM[:, 16 + oh:17 + oh]
                ex2 = SM[:, 18 + oh:19 + oh]
                var = SM[:, 20 + oh:21 + oh]
                sd = SM[:, 22 + oh:23 + oh]
                rstd = SM[:, 24 + oh:25 + oh]
                s1 = SM[:, 26 + oh:27 + oh]
                ms = SM[:, 28 + oh:29 + oh]
                s2 = SM[:, 30 + oh:31 + oh]
                nc.vector.tensor_scalar(mean, SM[:, 8 + oh:9 + oh],
                                        1.0 / NTOT, None, AluOpType.mult)
                nc.vector.tensor_scalar(ex2, SM[:, 10 + oh:11 + oh],
                                        1.0 / NTOT, None, AluOpType.mult)
                nc.vector.tensor_tensor(var, mean, mean, AluOpType.mult)
                nc.vector.tensor_tensor(var, ex2, var, AluOpType.subtract)
                nc.scalar.activation(sd, var, AF.Sqrt, bias=EPSAP)
                nc.vector.reciprocal(rstd, sd)
                nc.vector.tensor_tensor(s1, GBt[:, oh, 0:1], rstd, AluOpType.mult)
                nc.vector.tensor_tensor(ms, mean, s1, AluOpType.mult)
                nc.vector.tensor_tensor(s2, GBt[:, oh, 1:2], ms, AluOpType.subtract)
                OUTS = gtp.tile([128, HALF], F32, tag="g")
                nc.scalar.activation(OUTS[:], PSD[oh][:], AF.Relu,
                                     bias=s2, scale=s1)
                nc.sync.dma_start(out_d[128 * oh:128 * (oh + 1), :], OUTS[:])

    nc.compile()
    return nc


def host_inputs(x, w_off, b_off, w_def, gamma, beta):
    """Build the 8 per-core input dicts."""
    x = np.asarray(x, np.float32)
    w_off = np.asarray(w_off, np.float32)
    b_off = np.asarray(b_off, np.float32)
    w_def = np.asarray(w_def, np.float32)
    gamma = np.asarray(gamma, np.float32)
    beta = np.asarray(beta, np.float32)

    # weight stationaries, shared by all cores.
    # woff[cg, c, kk, j]: off-conv stationary for kernel position kk; output
    # column j<9 = tap j's dy channel (2j), j>=9 = tap (j-9)'s dx channel.
    woff = np.zeros((2, 128, K2, 18), np.float32)
    wdef = np.zeros((2, 128, K2, O), np.float32)
    for cg in range(2):
        cs = slice(128 * cg, 128 * (cg + 1))
        for kk in range(K2):
            ky, kx = kk // K, kk % K
            for j in range(K2):
                woff[cg, :, kk, j] = w_off[2 * j, cs, ky, kx]
                woff[cg, :, kk, 9 + j] = w_off[2 * j + 1, cs, ky, kx]
            wdef[cg, :, kk, :] = w_def[:, cs, ky, kx].T
    woff = woff.astype(ml_dtypes.bfloat16)
    wdef = wdef.astype(ml_dtypes.bfloat16)

    gb = np.zeros((2, 128, 2), np.float32)
    gb[0, :, 0], gb[1, :, 0] = gamma[:128], gamma[128:]
    gb[0, :, 1], gb[1, :, 1] = beta[:128], beta[128:]

    in_maps = []
    for core in range(NCORES):
        b, ph = core // 2, core % 2
        # extended zero-padded grid, rolled so ext rows [0..80) cover this
        # core's rows: ext row r corresponds to image row r - 8 + 32*ph
        xe = np.zeros((C, EG, EG), np.float32)
        r_lo, r_hi = 32 * ph - PADE, 32 * ph - PADE + EG
        s_lo, s_hi = max(0, r_lo), min(H, r_hi)
        xe[:, s_lo - r_lo:s_hi - r_lo, PADE:PADE + W] = x[b, :, s_lo:s_hi, :]
        # container pack for ap_gather: element i = (xe[i], xe[i+1])
        xcols = XTROWS + EG
        flatc = np.zeros((C, xcols + 1), ml_dtypes.bfloat16)
        flatc[:, PREB:PREB + EGF] = xe.reshape(C, EGF)
        lo = flatc[:, :xcols].view(np.uint16).astype(np.uint32)
        hi = flatc[:, 1:xcols + 1].view(np.uint16).astype(np.uint32)
        xp = (lo | (hi << np.uint32(16))).view(np.float32).reshape(2, 128, xcols)

        # kb[k, 0, p] = 16 + (ky-1) + h_local(p) + b_off_y ; h_local = p//64 + ...
        # NOTE: the gather/window row coords are *local* to the rolled grid:
        # local row of pixel p is p//64 (0..31), plus the conv sampling is
        # relative; py_local = off_y + (ky-1) + p//64.  The +16 mod-floor bias.
        kb = np.zeros((K2, 2, HALF), np.float32)
        pl = np.arange(HALF, dtype=np.float32)
        hloc = np.floor(pl / W)
        wloc = pl % W
        for kk in range(K2):
            ky, kx = kk // K, kk % K
            kb[kk, 0, :] = 15.5 + (ky - 1) + hloc + b_off[2 * kk]
            kb[kk, 1, :] = 15.5 + (kx - 1) + wloc + b_off[2 * kk + 1]
        m = {"woff": np.asarray(woff), "wdef": np.asarray(wdef),
             "kb": kb, "gb": gb, "xp": xp}
        in_maps.append(m)
    return in_maps


_prog_cache = {}


def _get_prog():
    if "nc" not in _prog_cache:
        _prog_cache["nc"] = build_program(NCORES)
    return _prog_cache["nc"]


def kernel(x, w_off, b_off, w_def, gamma, beta):
    nc = _get_prog()
    in_maps = host_inputs(x, w_off, b_off, w_def, gamma, beta)
    res = run_bass_kernel_spmd(nc, in_maps, core_ids=list(range(NCORES)))
    out = np.zeros((B, O, H, W), np.float32)
    for core in range(NCORES):
        b, ph = core // 2, core % 2
        out[b, :, 32 * ph:32 * (ph + 1), :] = \
            res.results[core]["out"].reshape(O, HROWS, W)
    return out



# revision 31
# speedup vs baseline: 8.3908x; 1.5918x over previous
"""Deformable-conv module (offset conv -> bilinear deform conv -> sync-BN -> ReLU)
as a Trainium2 Bass kernel on 8 NeuronCores.

Sharding: core = (batch b, pixel-half ph).  Each core computes the full
256-channel output for 2048 pixels (32 image rows) of one batch image.
Full C=256 contraction is local, so no partial-sum exchange is needed;
only BN statistics cross cores (one 4KB AllReduce).

Bilinear sampling: x is host-padded into an 80x80 zero-extended grid and
packed as bf16 (value, right-neighbor) pairs in fp32 containers.  One
ap_gather index fetches both x-corners of a row; the row+1 corners come
from the same index into the grid shifted by one row.  Zero padding makes
all out-of-image corners contribute exactly 0, so no validity masks are
needed.  The 4-corner bilinear sum is folded into the deform GEMM's
contraction (each corner stream is a moving operand; PSUM accumulates).
"""
import sys, os, time

sys.path.insert(0, "/opt/trn_rl_repo")

import numpy as np
import ml_dtypes

import concourse.bacc as bacc
import concourse.tile as tile
from concourse import mybir
from concourse import library_config
from concourse.alu_op_type import AluOpType
from concourse.bass_utils import run_bass_kernel_spmd

F32 = mybir.dt.float32
BF16 = mybir.dt.bfloat16
I16 = mybir.dt.int16
AF = mybir.ActivationFunctionType
AX = mybir.AxisListType
MM = mybir.MatmulPerfMode

B, C, H, W, O = 4, 256, 64, 64, 256
K = 3
K2 = 9
EPS = 1e-5
PADE = 8          # extension pad on each side of the image
EG = H + 2 * PADE  # 80: extended grid edge
EGF = EG * EG      # 6400 ext pixels
PREB = 648         # leading zero rows so biased indices need no -648
XTROWS = PREB + EGF  # 7048 rows in the HBM gather table
NELEM = 6967       # gather source rows (covers max biased index 6966)
MAGIC = 12582912.0  # 1.5 * 2**23: fp32 round-to-int trick
HALF = 2048        # pixels per core
HROWS = HALF // W  # 32 image rows per core
NCORES = 8
NTOT = B * H * W   # BN normalization count


def build_program(num_devices=NCORES):
    nc = bacc.Bacc("TRN2", target_bir_lowering=False, debug=False,
                   num_devices=num_devices, num_swdge_queues=4)

    xp_d = nc.dram_tensor("xp", [128, XTROWS + EG, 2], F32,
                          kind="ExternalInput").ap()
    woff_d = nc.dram_tensor("woff", [2, 128, K2, 18], BF16, kind="ExternalInput").ap()
    wdef_d = nc.dram_tensor("wdef", [2, 128, K2, O], BF16, kind="ExternalInput").ap()
    kb_d = nc.dram_tensor("kb", [K2, 2, HALF], F32, kind="ExternalInput").ap()
    gb_d = nc.dram_tensor("gb", [2, 128, 2], F32, kind="ExternalInput").ap()
    out_d = nc.dram_tensor("out", [O, HALF], F32, kind="ExternalOutput").ap()
    # internal DRAM scratch for layout bounces
    widx_d = nc.dram_tensor("widx_s", [K2, HALF], I16).ap()
    wmap_d = nc.dram_tensor("wmap_s", [K2, 2, HALF], F32).ap()
    stats_in_d = nc.dram_tensor("stats_in", [128, 4], F32).ap()
    stats_out_d = nc.dram_tensor("stats_out", [128, 4], F32, addr_space="Shared").ap()

    with tile.TileContext(nc) as tc:
        with tc.tile_pool(name="per", bufs=1) as per, \
             tc.tile_pool(name="wb", bufs=2) as wbp, \
             tc.tile_pool(name="gt", bufs=2) as gtp, \
             tc.tile_pool(name="tt", bufs=2) as ttp, \
             tc.tile_pool(name="ps", bufs=2, space="PSUM") as psp:

            # ---- inputs to SBUF ----
            # d=2 gather table: element i = (cg0 container, cg1 container),
            # each container = bf16 (xe[i], xe[i+1]) pair in an fp32 word.
            # One gather request then fetches both channel groups, halving the
            # Q7 read-request count that rate-limits ap_gather.
            nc.gpsimd.load_library(library_config.ap_gather)
            XP2 = per.tile([128, XTROWS + EG, 2], F32, tag="xp2")
            nc.sync.dma_start(XP2[:], xp_d[:])
            WOFF = per.tile([128, 2, K2, 18], BF16, tag="woff")
            nc.sync.dma_start(WOFF[:, 0], woff_d[0])
            nc.sync.dma_start(WOFF[:, 1], woff_d[1])
            WDEF = per.tile([128, 2, K2, O], BF16, tag="wdef")
            nc.sync.dma_start(WDEF[:, 0], wdef_d[0])
            nc.sync.dma_start(WDEF[:, 1], wdef_d[1])
            GBt = per.tile([128, 2, 2], F32, tag="gb")
            nc.sync.dma_start(GBt[:, 0], gb_d[0])
            nc.sync.dma_start(GBt[:, 1], gb_d[1])

            # [part, 80 ext rows, 80 cols] view for conv windows: stride-4
            # bf16 view into the packed table (bf16 4i+2cg = xe_cg[i])
            xvb = XP2[:, PREB:PREB + EGF].bitcast(BF16)\
                .rearrange("p (r c) e -> p r (c e)", c=EG)
            xv = [xvb[:, :, 2 * cg::4] for cg in range(2)]
            xstep = 1

            # ---- offset conv: two GEMMs (y comps, x comps) ----
            # psum partitions = 9 taps; moving = 32x64 image window, 512-col chunks
            ps_y = psp.tile([K2, HALF], F32, tag="ps")
            ps_x = psp.tile([K2, HALF], F32, tag="ps")
            n_mm = 0
            for comp, pst in ((0, ps_y), (1, ps_x)):
                for cg in range(2):
                    for kk in range(K2):
                        dy, dx = kk // K - 1, kk % K - 1
                        r0 = PADE + dy   # ext row of first window row (ph folded in kb? no: rows differ per core!)
                        # NOTE: per-core row offset handled via kb; but the
                        # window itself must read this core's rows.  The row
                        # base depends on ph which is NOT known at build time,
                        # so we bake it via a per-core DRAM input instead?  No:
                        # all cores run the same program; we make the window
                        # row base a *program constant* = PADE+dy and add the
                        # per-core 32-row offset by shifting the data on host?
                        # Host shifts: xp is the full 80x80 grid; instead the
                        # moving AP below uses rows [PADE+dy+ROWOFF ...] with
                        # ROWOFF supplied via host-rolled kb... -> resolved by
                        # building xp PER CORE with the core's 32-row window
                        # centered: see host prep (xe rolled so that rows
                        # [PADE..PADE+32) are this core's rows).
                        stat = WOFF[:, cg, kk, 9 * comp:9 * comp + 9]
                        for q in range(4):
                            rq = r0 + 8 * q
                            mov = xv[cg][:, rq:rq + 8,
                                         xstep * (PADE + dx):
                                         xstep * (PADE + dx) + xstep * W:xstep]
                            nc.tensor.matmul(
                                pst[:, 512 * q:512 * (q + 1)], stat, mov,
                                start=(cg == 0 and kk == 0),
                                stop=(cg == 1 and kk == K2 - 1))
                            n_mm += 1

            # ---- coordinate math ----
            # A = py + 15.5 (kb holds tap offset + base + 15.5); add conv psums
            AYX = per.tile([K2, 2, HALF], F32, tag="ayx")
            nc.sync.dma_start(AYX[:], kb_d[:])
            nc.vector.tensor_tensor(AYX[:, 0], AYX[:, 0], ps_y[:], AluOpType.add)
            nc.vector.tensor_tensor(AYX[:, 1], AYX[:, 1], ps_x[:], AluOpType.add)
            # FI = round(A) = floor(py) + 16 (fp32 magic-number round)
            FI = per.tile([K2, 2, HALF], F32, tag="fi")
            nc.vector.tensor_scalar(FI[:], AYX[:], MAGIC, -MAGIC,
                                    AluOpType.add, AluOpType.add)
            # D = A - FI in (-0.5, 0.5]; frac = D + 0.5, 1-frac = 0.5 - D
            nc.vector.tensor_tensor(AYX[:], AYX[:], FI[:], AluOpType.subtract)
            # clip FI to [8, 86] (= coord in [-8, 70]), in place
            nc.vector.tensor_scalar(FI[:], FI[:], 8.0, 86.0,
                                    AluOpType.max, AluOpType.min)
            # gather index = 80*ycl + xcl (maps into the PREB-padded grid)
            nc.vector.scalar_tensor_tensor(FI[:, 1], FI[:, 0], 80.0, FI[:, 1],
                                           AluOpType.mult, AluOpType.add)
            WIDX16 = per.tile([K2, HALF], I16, tag="widx16")
            nc.vector.tensor_copy(WIDX16[:], FI[:, 1])
            # OM = 0.5 - D = 1 - frac
            OM = per.tile([K2, 2, HALF], F32, tag="om")
            nc.vector.tensor_scalar(OM[:], AYX[:], -1.0, 0.5,
                                    AluOpType.mult, AluOpType.add)
            # FI[:,1] now free: reuse for frac_x = D_x + 0.5
            nc.vector.tensor_scalar(FI[:, 1], AYX[:, 1], 0.5, None, AluOpType.add)
            # pixel-interleaved weight maps [k, row, (pixel, x-corner)] so the
            # per-iteration broadcast DMA is contiguous; the bf16 (w_x0, w_x1)
            # pair of a pixel shares one fp32 container word
            WMI = per.tile([K2, 2, 2 * HALF], BF16, tag="wmi")
            nc.vector.tensor_tensor(WMI[:, 0, 0::2], OM[:, 0], OM[:, 1],
                                    AluOpType.mult)
            nc.vector.tensor_tensor(WMI[:, 0, 1::2], OM[:, 0], FI[:, 1],
                                    AluOpType.mult)
            nc.vector.scalar_tensor_tensor(WMI[:, 1, 0::2], AYX[:, 0], 0.5,
                                           OM[:, 1], AluOpType.add,
                                           AluOpType.mult)
            nc.vector.scalar_tensor_tensor(WMI[:, 1, 1::2], AYX[:, 0], 0.5,
                                           FI[:, 1], AluOpType.add,
                                           AluOpType.mult)

            # ---- DRAM bounce: wrap indices for ap_gather, broadcast weights ----
            nc.sync.dma_start(widx_d[:], WIDX16[:])
            nc.sync.dma_start(wmap_d[:], WMI.bitcast(F32)[:])
            WIDXW = per.tile([128, K2, HALF // 16], I16, tag="widxw")
            widx_r = widx_d.rearrange("k (c s) -> s k c", s=16)
            for g in range(8):
                nc.sync.dma_start(WIDXW[16 * g:16 * (g + 1)], widx_r)

            # ---- main loop: gather, weight, GEMM-accumulate ----
            PSD0 = psp.tile([128, HALF], F32, tag="ps")
            PSD1 = psp.tile([128, HALF], F32, tag="ps")
            PSD = [PSD0, PSD1]
            for kk in range(K2):
                for row in range(2):
                    # contiguous broadcast of the pixel-interleaved pair weights
                    WB = wbp.tile([128, HALF], F32, tag="wb")
                    nc.sync.dma_start(WB[:], wmap_d[kk, row].unsqueeze(0)
                                      .broadcast_to((128, HALF)))
                    # one d=2 gather fetches both channel groups per index
                    G = gtp.tile([128, HALF, 2], F32, tag="g")
                    nc.gpsimd.ap_gather(
                        G[:], XP2[:, EG * row:EG * row + NELEM + 33],
                        WIDXW[:, kk], channels=128, num_elems=NELEM + 33,
                        d=2, num_idxs=HALF)
                    Gb = G.bitcast(BF16)  # [128, HALF, 4]
                    for cg in range(2):
                        T = ttp.tile([128, 2 * HALF], BF16, tag="t")
                        nc.vector.tensor_tensor(
                            T.rearrange("p (i e) -> p i e", e=2)[:],
                            Gb[:, :, 2 * cg:2 * cg + 2],
                            WB.bitcast(BF16).rearrange("p (i e) -> p i e",
                                                       e=2)[:],
                            AluOpType.mult)
                        first = (kk == 0 and row == 0 and cg == 0)
                        last = (kk == K2 - 1 and row == 1 and cg == 1)
                        for oh in range(2):
                            stat = WDEF[:, cg, kk, 128 * oh:128 * (oh + 1)]
                            for par in range(2):
                                Tp = T[:, par::2]
                                for q in range(4):
                                    mov = Tp[:, 512 * q:512 * (q + 1)]
                                    nc.tensor.matmul(
                                        PSD[oh][:, 512 * q:512 * (q + 1)],
                                        stat, mov,
                                        start=(first and par == 0),
                                        stop=(last and par == 1))

            # ---- BN stats + AllReduce ----
            SM = per.tile([128, 48], F32, tag="sm")
            TRASH = ttp.tile([128, 2 * HALF], BF16, tag="t")
            ZERO = SM[:, 40:41]
            EPSAP = SM[:, 41:42]
            nc.vector.memset(ZERO, 0.0)
            nc.vector.memset(EPSAP, float(EPS))
            for oh in range(2):
                nc.vector.tensor_reduce(SM[:, oh:oh + 1], PSD[oh][:],
                                        AX.X, AluOpType.add)
                nc.scalar.activation(TRASH[:, 0:HALF], PSD[oh][:], AF.Square,
                                     bias=ZERO, accum_out=SM[:, 2 + oh:3 + oh])
            nc.sync.dma_start(stats_in_d[:], SM[:, 0:4])
            nc.gpsimd.collective_compute(
                "AllReduce", AluOpType.add,
                replica_groups=[list(range(num_devices))],
                ins=[stats_in_d[:]], outs=[stats_out_d[:]])
            nc.sync.dma_start(SM[:, 8:12], stats_out_d[:])

            for oh in range(2):
                mean = SM[:, 16 + oh:17 + oh]
                ex2 = SM[:, 18 + oh:19 + oh]
                var = SM[:, 20 + oh:21 + oh]
                sd = SM[:, 22 + oh:23 + oh]
                rstd = SM[:, 24 + oh:25 + oh]
                s1 = SM[:, 26 + oh:27 + oh]
                ms = SM[:, 28 + oh:29 + oh]
                s2 = SM[:, 30 + oh:31 + oh]
                nc.vector.tensor_scalar(mean, SM[:, 8 + oh:9 + oh],
                                        1.0 / NTOT, None, AluOpType.mult)
                nc.vector.tensor_scalar(ex2, SM[:, 10 + oh:11 + oh],
                                        1.0 / NTOT, None, AluOpType.mult)
                nc.vector.tensor_tensor(var, mean, mean, AluOpType.mult)
                nc.vector.tensor_tensor(var, ex2, var, AluOpType.subtract)
                nc.scalar.activation(sd, var, AF.Sqrt, bias=EPSAP)
                nc.vector.reciprocal(rstd, sd)
                nc.vector.tensor_tensor(s1, GBt[:, oh, 0:1], rstd, AluOpType.mult)
                nc.vector.tensor_tensor(ms, mean, s1, AluOpType.mult)
                nc.vector.tensor_tensor(s2, GBt[:, oh, 1:2], ms, AluOpType.subtract)
                OUTS = gtp.tile([128, HALF], F32, tag="g")
                nc.scalar.activation(OUTS[:], PSD[oh][:], AF.Relu,
                                     bias=s2, scale=s1)
                nc.sync.dma_start(out_d[128 * oh:128 * (oh + 1), :], OUTS[:])

    nc.compile()
    return nc


def host_inputs(x, w_off, b_off, w_def, gamma, beta):
    """Build the 8 per-core input dicts."""
    x = np.asarray(x, np.float32)
    w_off = np.asarray(w_off, np.float32)
    b_off = np.asarray(b_off, np.float32)
    w_def = np.asarray(w_def, np.float32)
    gamma = np.asarray(gamma, np.float32)
    beta = np.asarray(beta, np.float32)

    # weight stationaries, shared by all cores.
    # woff[cg, c, kk, j]: off-conv stationary for kernel position kk; output
    # column j<9 = tap j's dy channel (2j), j>=9 = tap (j-9)'s dx channel.
    woff = np.zeros((2, 128, K2, 18), np.float32)
    wdef = np.zeros((2, 128, K2, O), np.float32)
    for cg in range(2):
        cs = slice(128 * cg, 128 * (cg + 1))
        for kk in range(K2):
            ky, kx = kk // K, kk % K
            for j in range(K2):
                woff[cg, :, kk, j] = w_off[2 * j, cs, ky, kx]
                woff[cg, :, kk, 9 + j] = w_off[2 * j + 1, cs, ky, kx]
            wdef[cg, :, kk, :] = w_def[:, cs, ky, kx].T
    woff = woff.astype(ml_dtypes.bfloat16)
    wdef = wdef.astype(ml_dtypes.bfloat16)

    gb = np.zeros((2, 128, 2), np.float32)
    gb[0, :, 0], gb[1, :, 0] = gamma[:128], gamma[128:]
    gb[0, :, 1], gb[1, :, 1] = beta[:128], beta[128:]

    in_maps = []
    for core in range(NCORES):
        b, ph = core // 2, core % 2
        # extended zero-padded grid, rolled so ext rows [0..80) cover this
        # core's rows: ext row r corresponds to image row r - 8 + 32*ph
        xe = np.zeros((C, EG, EG), np.float32)
        r_lo, r_hi = 32 * ph - PADE, 32 * ph - PADE + EG
        s_lo, s_hi = max(0, r_lo), min(H, r_hi)
        xe[:, s_lo - r_lo:s_hi - r_lo, PADE:PADE + W] = x[b, :, s_lo:s_hi, :]
        # container pack for ap_gather: element i = (xe[i], xe[i+1])
        xcols = XTROWS + EG
        flatc = np.zeros((C, xcols + 1), ml_dtypes.bfloat16)
        flatc[:, PREB:PREB + EGF] = xe.reshape(C, EGF)
        lo = flatc[:, :xcols].view(np.uint16).astype(np.uint32)
        hi = flatc[:, 1:xcols + 1].view(np.uint16).astype(np.uint32)
        xpf = (lo | (hi << np.uint32(16))).view(np.float32).reshape(2, 128, xcols)
        # d=2 element = (cg0 container, cg1 container)
        xp = np.ascontiguousarray(np.stack([xpf[0], xpf[1]], axis=-1))

        # kb[k, 0, p] = 16 + (ky-1) + h_local(p) + b_off_y ; h_local = p//64 + ...
        # NOTE: the gather/window row coords are *local* to the rolled grid:
        # local row of pixel p is p//64 (0..31), plus the conv sampling is
        # relative; py_local = off_y + (ky-1) + p//64.  The +16 mod-floor bias.
        kb = np.zeros((K2, 2, HALF), np.float32)
        pl = np.arange(HALF, dtype=np.float32)
        hloc = np.floor(pl / W)
        wloc = pl % W
        for kk in range(K2):
            ky, kx = kk // K, kk % K
            kb[kk, 0, :] = 15.5 + (ky - 1) + hloc + b_off[2 * kk]
            kb[kk, 1, :] = 15.5 + (kx - 1) + wloc + b_off[2 * kk + 1]
        m = {"woff": np.asarray(woff), "wdef": np.asarray(wdef),
             "kb": kb, "gb": gb, "xp": xp}
        in_maps.append(m)
    return in_maps


_prog_cache = {}


def _get_prog():
    if "nc" not in _prog_cache:
        _prog_cache["nc"] = build_program(NCORES)
    return _prog_cache["nc"]


def kernel(x, w_off, b_off, w_def, gamma, beta):
    nc = _get_prog()
    in_maps = host_inputs(x, w_off, b_off, w_def, gamma, beta)
    res = run_bass_kernel_spmd(nc, in_maps, core_ids=list(range(NCORES)))
    out = np.zeros((B, O, H, W), np.float32)
    for core in range(NCORES):
        b, ph = core // 2, core % 2
        out[b, :, 32 * ph:32 * (ph + 1), :] = \
            res.results[core]["out"].reshape(O, HROWS, W)
    return out

